# revision 14
# baseline (speedup 1.0000x reference)
"""Trainium2 Bass kernel for nn_AutoformerBase (sparse_attention).

Algorithm (algebraically reduced from the reference):
  mean_value[b, tau] = (1/D) sum_d corr(q_proj_d, k_proj_d)[tau]
                     = sum_{t,j} rho[b,t,j] * k_in[b,(t-tau)%L,j],
    where rho = q_in @ A and A = (Wq @ Wk^T)/D   (bq/bk only shift
    mean_value by a per-batch constant -> no effect on top-k or softmax).
  index = top6 of mean over b of mean_value  (one tiny AllReduce)
  tmp_corr = softmax(mean_value[:, index])
  out = sum_k tmp_corr[:,k] * Z[:, (t+tau_k)%L, :] + (bv@Wo + bo),
    where Z = v_in @ (Wv @ Wo)  (rolls commute with the right-side matmul,
    softmax weights sum to 1 for the bias term).

Device strategy (per core, data-parallel over batch, 4 batches/core):
  - bf16 matmuls (fp8 measured numerically unusable here: top-6 gap in the
    batch-mean is 0.004 = 8% of sigma; fp8 stats noise ~0.05 absolute).
  - Measured PE behavior drives the structure: 512-col matmuls stream at
    ~260ns (bank switches / acc-group starts cost ~43/172ns; sub-260 needs
    >8-deep acc groups, unreachable at contraction 512). Any wait drops
    the PE to 1.2GHz for ~3us (pstate ramp), so PSUM rotates 4 gens deep
    and the mean_value accumulator lives in SBUF (folds borrow a rotating
    PSUM gen, evac'd by scalar, accumulated by gpsimd).
  - mean_value via Gram diag-sums: G row-blocks evacuated PSUM->fp16,
    shear-DMA'd so piece-buffer columns hold circulant diagonals, fp16
    2x-mode DVE adds into [128, 2L] tau-aligned accumulators, 4-matmul
    fold per batch (rows 0-3 = per-batch mean_value, row 32 = total).
  - AllReduce of the (L,) total triggered right after the last fold; the
    whole Z = vT @ (WvWo) phase runs after the trigger to cover the
    collective (intrinsic ~13us + inter-core skew). A warmup AllReduce at
    kernel start absorbs CC-channel setup.
  - Aggregation tail split three ways: PE weighted-identity matmuls over
    a doubled Z, a scalar-engine scale-pipe feeding static 2x-mode DVE
    adds, and DVE-solo dynamic-offset tiles; out-DMAs round-robin.
"""
import math
from contextlib import ExitStack
import numpy as np
from ml_dtypes import bfloat16

import concourse.bass as bass
import concourse.mybir as mybir
import concourse.tile as tile
from concourse import bacc
from concourse.bass import ds
from concourse.tile import TileContext
from concourse.bass_utils import run_bass_kernel_spmd

B, L, D = 32, 1024, 512
NCORES = 8
BLOC = B // NCORES          # 4 batches per core
TOPK = 6
F32 = mybir.dt.float32
BF16 = mybir.dt.bfloat16
FP16 = mybir.dt.float16
U32 = mybir.dt.uint32
ALU = mybir.AluOpType
AFT = mybir.ActivationFunctionType

_CACHE = {}

NIC = D // 128           # 4 chunks of contraction
NJC = D // 128           # 4 chunks of output-feature rows
NTC = L // 512           # 2 free-dim chunks of 512
NTR = L // 128           # 8 row-blocks of t'
NPB = 4                  # rotating sheared piece buffers

# aggregation tile split: PE identity-matmuls / DVE dynamic-offset tiles
AGG_PAT = ['pe', 'pe', 'pe', 'dve'] * 8


def _pb_base(tr):
    """Column offset of piece tr's sheared band inside pbacc[128, 2048]:
    pbacc column u = base_tr + kappa holds tau = u mod L contributions."""
    return (-128 - 128 * tr) % L


def _build():
    nc = bacc.Bacc("TRN2", target_bir_lowering=False)

    qT_d = nc.dram_tensor("qT", [BLOC, D, L], BF16, kind="ExternalInput")
    kT_d = nc.dram_tensor("kT", [BLOC, D, L], BF16, kind="ExternalInput")
    vT_d = nc.dram_tensor("vT", [BLOC, D, L], BF16, kind="ExternalInput")
    A_d = nc.dram_tensor("A", [D, D], BF16, kind="ExternalInput")
    Wc_d = nc.dram_tensor("Wc", [D, D], BF16, kind="ExternalInput")
    I_d = nc.dram_tensor("I128", [128, 128], BF16, kind="ExternalInput")
    selrow_d = nc.dram_tensor("selrow", [4, BLOC * 128], BF16,
                              kind="ExternalInput")
    outT_d = nc.dram_tensor("outT", [BLOC, D, L], BF16, kind="ExternalOutput")
    cc_in = nc.dram_tensor("cc_in", [1, L], F32)
    cc_out = nc.dram_tensor("cc_out", [1, L], F32, addr_space="Shared")
    cw_in = nc.dram_tensor("cw_in", [1, 8], F32)
    cw_out = nc.dram_tensor("cw_out", [1, 8], F32, addr_space="Shared")

    with TileContext(nc) as tc, ExitStack() as ctx:
        consts = ctx.enter_context(tc.tile_pool(name="consts", bufs=1))
        qk_pool = ctx.enter_context(tc.tile_pool(name="qk", bufs=2))
        rho_pool = ctx.enter_context(tc.tile_pool(name="rho", bufs=2))
        vz_pool = ctx.enter_context(tc.tile_pool(name="vz", bufs=1))
        gst_pool = ctx.enter_context(tc.tile_pool(name="gst", bufs=3))
        pb_pool = ctx.enter_context(tc.tile_pool(name="pb", bufs=1))
        small = ctx.enter_context(tc.tile_pool(name="small", bufs=1))
        out_pool = ctx.enter_context(tc.tile_pool(name="out", bufs=3))
        term_pool = ctx.enter_context(tc.tile_pool(name="term", bufs=2))
        ps_big = ctx.enter_context(tc.tile_pool(name="ps_big", bufs=4,
                                                space="PSUM"))

        # warmup collective: absorbs CC channel setup + initial skew on the
        # CC stream, independent of all compute
        wz = small.tile([1, 8], F32)
        nc.vector.memset(wz, 0.0)
        nc.gpsimd.dma_start(out=cw_in.ap(), in_=wz)
        nc.gpsimd.collective_compute(
            "AllReduce", ALU.add,
            replica_groups=[list(range(NCORES))],
            ins=[cw_in.ap()], outs=[cw_out.ap()])

        # ---- constants ----
        A_sb = consts.tile([128, NIC, D], BF16)
        Wc_sb = consts.tile([128, NIC, D], BF16)
        I_sb = consts.tile([128, 128], BF16)

        # fold stationaries: sel[:, b, :] is [128, 36] with column b = ones
        # (batch b's diag-sums on psum partition b) and column 32 = ones
        # (batch TOTAL on psum partition 32; 32-aligned for APs).
        sel = consts.tile([128, BLOC, 36], FP16)
        nc.vector.memset(sel, 0.0)
        for b in range(BLOC):
            nc.vector.memset(sel[:, b, b:b + 1], 1.0)
            nc.vector.memset(sel[:, b, 32:33], 1.0)
        selrow = consts.tile([4, BLOC, 128], BF16)

        # persistent sheared piece buffers: borders stay zero across reuse
        pbs = []
        for i in range(NPB):
            pb_t = pb_pool.tile([128, 1152], FP16, tag=f"pb{i}", name=f"pb{i}")
            nc.vector.memset(pb_t, 0.0)
            pbs.append(pb_t)
        pbaccs = []
        for i in range(2):
            pa = pb_pool.tile([128, 2 * L], FP16, tag=f"pbacc{i}",
                              name=f"pbacc{i}")
            nc.vector.memset(pa, 0.0)
            pbaccs.append(pa)

        # SBUF mean_value accumulator (rows 0-3 per-batch, row 32 total)
        delta_sb = small.tile([36, L], F32)

        # ---- input DMAs ----
        # sync queue: A/q(b0) interleaved by chunk so rho starts earliest,
        # then q(b1), pair-1 q/k, Wc/I/selrow, vT.
        # scalar queue: k(b0), k(b1) only (free before shears start).
        qTs, kTs, vTs = {}, {}, {}
        for b in range(BLOC):
            for ic in range(NIC):
                qTs[(b, ic)] = qk_pool.tile([128, L], BF16, tag=f"q{ic}",
                                            name=f"qT{b}_{ic}")
                kTs[(b, ic)] = qk_pool.tile([128, L], BF16, tag=f"k{ic}",
                                            name=f"kT{b}_{ic}")

        def dma_in(t_d, ts, b, ic, eng):
            eng.dma_start(out=ts[(b, ic)],
                          in_=t_d.ap()[b, ic * 128:(ic + 1) * 128])

        for ic in range(NIC):
            nc.sync.dma_start(
                out=A_sb[:, ic, :],
                in_=A_d.ap()[ic * 128:(ic + 1) * 128, :])
            dma_in(qT_d, qTs, 0, ic, nc.sync)
        for ic in range(NIC):
            dma_in(qT_d, qTs, 1, ic, nc.sync)
        for b in (0, 1):
            for ic in range(NIC):
                dma_in(kT_d, kTs, b, ic, nc.scalar)
        for b in range(BLOC):
            vTs[b] = vz_pool.tile([128, NIC, L], BF16, tag=f"v{b}",
                                  name=f"vT{b}")

        # ================= stats phase =================
        pb_i = [0]
        piece_i = [0]
        deferred_add = []   # (b, tr, pb)
        pending_folds = []
        fold_first = [True]

        def emit_adds(depth=2):
            while len(deferred_add) > depth:
                b, tr, pb = deferred_add.pop(0)
                base = _pb_base(tr)
                pa = pbaccs[b % 2]
                nc.vector.tensor_tensor(
                    out=pa[:, base:base + 1152],
                    in0=pb, in1=pa[:, base:base + 1152], op=ALU.add)

        def emit_fold(b):
            # fold pbacc into a rotating psum gen (rows 0-35), evac to
            # SBUF staging, accumulate into delta_sb on gpsimd
            pa = pbaccs[b % 2]
            ps = ps_big.tile([128, L], F32, tag="pa", name=f"fps{b}")
            for seg in range(4):
                h = seg % 2
                nc.tensor.matmul(
                    ps[0:36, h * 512:(h + 1) * 512],
                    lhsT=sel[:, b, :],
                    rhs=pa[:, seg * 512:(seg + 1) * 512],
                    start=(seg < 2), stop=(seg >= 2),
                    skip_group_check=True)
            nc.vector.memset(pa, 0.0)
            fst = term_pool.tile([36, L], FP16, tag="fold", name=f"fsb{b}")
            nc.scalar.copy(fst, ps[0:36, :])
            if fold_first[0]:
                nc.gpsimd.tensor_copy(delta_sb, fst)
                fold_first[0] = False
            else:
                nc.gpsimd.tensor_tensor(out=delta_sb, in0=fst, in1=delta_sb,
                                        op=ALU.add)

        def emit_rho(b):
            rho = rho_pool.tile([128, NJC, L], BF16, tag="rho", name=f"rho{b}")
            for jc in range(NJC):
                ps = ps_big.tile([128, L], F32, tag="pa", name=f"rps{b}_{jc}")
                for ic in range(NIC):
                    for tcc in range(NTC):
                        nc.tensor.matmul(
                            ps[:, tcc * 512:(tcc + 1) * 512],
                            lhsT=A_sb[:, ic, jc * 128:(jc + 1) * 128],
                            rhs=qTs[(b, ic)][:, tcc * 512:(tcc + 1) * 512],
                            start=(ic == 0), stop=(ic == NIC - 1),
                            skip_group_check=True)
                nc.scalar.copy(rho[:, jc, 0:512], ps[:, 0:512])
                nc.vector.tensor_copy(rho[:, jc, 512:L], ps[:, 512:L])
                if jc >= 2 and pending_folds:
                    emit_fold(pending_folds.pop(0))
            return rho

        def emit_gram_rb(b, rho, tr):
            ps = ps_big.tile([128, L], F32, tag="pa", name=f"gps{b}_{tr}")
            for jc in range(NJC):
                for tcc in range(NTC):
                    nc.tensor.matmul(
                        ps[:, tcc * 512:(tcc + 1) * 512],
                        lhsT=kTs[(b, jc)][:, tr * 128:(tr + 1) * 128],
                        rhs=rho[:, jc, tcc * 512:(tcc + 1) * 512],
                        start=(jc == 0), stop=(jc == NJC - 1),
                        skip_group_check=True)
            emit_adds(depth=2)
            gst = gst_pool.tile([128, L], FP16, tag="gst")
            nc.scalar.copy(gst[:, 0:512], ps[:, 0:512])
            nc.vector.tensor_copy(gst[:, 512:L], ps[:, 512:L])
            pb = pbs[pb_i[0] % NPB]
            pb_i[0] += 1
            shear = bass.AP(tensor=pb.tensor, offset=pb.offset + 128,
                            ap=[[1152 - 1, 128], [1, L]])
            nc.gpsimd.dma_start(out=shear, in_=gst)
            piece_i[0] += 1
            deferred_add.append((b, tr, pb))

        for b0, b1 in ((0, 1), (2, 3)):
            emit_adds(depth=0)  # all prior-pair adds before folds read pbacc
            rhos = {b0: emit_rho(b0), b1: emit_rho(b1)}
            for b in (b0, b1):
                for tr in range(NTR):
                    emit_gram_rb(b, rhos[b], tr)
            pending_folds.extend([b0, b1])
            if b0 == 0:
                for b in (2, 3):
                    for ic in range(NIC):
                        dma_in(kT_d, kTs, b, ic, nc.scalar)
                for b in (2, 3):
                    for ic in range(NIC):
                        dma_in(qT_d, qTs, b, ic, nc.scalar)
                nc.scalar.dma_start(out=Wc_sb, in_=Wc_d.rearrange(
                    "(ic p) j -> p ic j", p=128))
                nc.scalar.dma_start(out=I_sb, in_=I_d.ap())
                nc.scalar.dma_start(out=selrow, in_=selrow_d.rearrange(
                    "p (b c) -> p b c", b=BLOC))
                for b in range(BLOC):
                    nc.scalar.dma_start(out=vTs[b], in_=vT_d.ap()[b].rearrange(
                        "(ic p) t -> p ic t", p=128))

        emit_adds(depth=0)

        # ================= Z phase (covers the AllReduce) =================
        z_tiles = {}
        for b in range(BLOC):
            z_tiles[b] = vz_pool.tile([128, NJC, 2 * L], BF16, tag=f"z{b}",
                                      name=f"Z{b}")

        def emit_z_group(b0, b1, jc, tcc):
            ps = ps_big.tile([128, L], F32, tag="pa", name=f"zps{b0}_{jc}_{tcc}")
            pss = [ps[:, 0:512], ps[:, 512:L]]
            for ic in range(NIC):
                for i, b in enumerate((b0, b1)):
                    nc.tensor.matmul(
                        pss[i],
                        lhsT=Wc_sb[:, ic, jc * 128:(jc + 1) * 128],
                        rhs=vTs[b][:, ic, tcc * 512:(tcc + 1) * 512],
                        start=(ic == 0), stop=(ic == NIC - 1),
                        skip_group_check=True)
            for i, b in enumerate((b0, b1)):
                dst = z_tiles[b][:, jc, tcc * 512:(tcc + 1) * 512]
                if i == 0:
                    nc.scalar.copy(dst, pss[i])
                else:
                    nc.vector.tensor_copy(dst, pss[i])

        # first two Z groups hide the final folds, then the AR triggers
        emit_z_group(0, 1, 0, 0)
        emit_fold(pending_folds.pop(0))
        emit_z_group(0, 1, 0, 1)
        emit_fold(pending_folds.pop(0))

        nc.sync.dma_start(out=cc_in.ap(), in_=delta_sb[32:33, :])
        nc.gpsimd.collective_compute(
            "AllReduce", ALU.add,
            replica_groups=[list(range(NCORES))],
            ins=[cc_in.ap()], outs=[cc_out.ap()])
        bm = small.tile([1, L], F32)
        nc.sync.dma_start(out=bm, in_=cc_out.ap())
        mv4 = delta_sb[0:4, :]

        for b0, b1 in ((0, 1), (2, 3)):
            for jc in range(NJC):
                for tcc in range(NTC):
                    if b0 == 0 and jc == 0:
                        continue
                    emit_z_group(b0, b1, jc, tcc)
            for b in (b0, b1):
                nc.gpsimd.dma_start(out=z_tiles[b][:, :, L:2 * L],
                                    in_=z_tiles[b][:, :, 0:L])

        # ================= top-k + weights =================
        vals8 = small.tile([1, 8], F32)
        idx8 = small.tile([1, 8], U32)
        nc.vector.max_with_indices(vals8, idx8, bm)

        def tau_regs(eng, pfx):
            regs = []
            for k in range(TOPK):
                r = eng.alloc_register(f"{pfx}{k}")
                eng.reg_load(r, idx8[0:1, k:k + 1])
                regs.append(nc.snap(r, min_val=0, max_val=L - 1))
            return regs

        tau_t = tau_regs(nc.tensor, "tau_t")
        tau_v = tau_regs(nc.vector, "tau_v")

        # gather mv4[:, tau_k]; softmax without max-subtraction (mean_value
        # magnitudes are O(1); exp is safe in fp32)
        w4 = small.tile([4, 8], F32)
        for k in range(TOPK):
            nc.vector.tensor_copy(w4[:, k:k + 1], mv4[:, ds(tau_v[k], 1)])
        ex = small.tile([4, 8], F32)
        sm = small.tile([4, 1], F32)
        nc.scalar.activation(ex[:, 0:TOPK], w4[:, 0:TOPK], AFT.Exp,
                             accum_out=sm)
        rc = small.tile([4, 1], F32)
        nc.vector.reciprocal(rc, sm)
        wnb = small.tile([4, 8], BF16)
        nc.vector.memset(wnb, 0.0)
        nc.vector.tensor_scalar(out=wnb[:, 0:TOPK], in0=ex[:, 0:TOPK],
                                scalar1=rc, scalar2=None, op0=ALU.mult)

        # broadcast w to all 128 partitions: w_bc[p, b*8+k] = w[b, k]
        psw = ps_big.tile([128, L], F32, tag="pa", name="psw")
        for b in range(BLOC):
            nc.tensor.matmul(psw[:, b * 8:(b + 1) * 8], lhsT=selrow[:, b, :],
                             rhs=wnb, start=True, stop=True,
                             skip_group_check=True)
        w_bc = small.tile([128, BLOC, 8], F32)
        nc.scalar.copy(w_bc, psw[:, 0:BLOC * 8])

        # weighted identities for the PE aggregation path
        wIs = {}
        for b in range(BLOC):
            wIs[b] = []
            for k in range(TOPK):
                t = small.tile([128, 128], BF16, tag=f"wI{b}_{k}",
                               name=f"wI{b}_{k}")
                nc.scalar.activation(t, I_sb, AFT.Copy,
                                     scale=w_bc[:, b, k:k + 1])
                wIs[b].append(t)

        # ================= aggregation =================
        tiles = [(b, jc, tcc) for b in range(BLOC)
                 for jc in range(NJC) for tcc in range(NTC)]
        dma_i = [0]

        def out_dma(b, jc, tcc, acc):
            eng = (nc.gpsimd, nc.scalar)[dma_i[0] % 2]
            dma_i[0] += 1
            eng.dma_start(
                out=outT_d.ap()[b, jc * 128:(jc + 1) * 128,
                                tcc * 512:(tcc + 1) * 512],
                in_=acc)

        def agg_dve(b, jc, tcc):
            Z = z_tiles[b]
            Zw = Z if tcc == 0 else Z[:, :, 512:2 * L]
            acc = out_pool.tile([128, 512], BF16, tag="dacc",
                                name=f"vacc{b}_{jc}_{tcc}")
            nc.vector.tensor_scalar(out=acc, in0=Zw[:, jc, ds(tau_v[0], 512)],
                                    scalar1=w_bc[:, b, 0:1], scalar2=None,
                                    op0=ALU.mult)
            for k in range(1, TOPK):
                nc.vector.scalar_tensor_tensor(
                    out=acc, in0=Zw[:, jc, ds(tau_v[k], 512)],
                    scalar=w_bc[:, b, k:k + 1], in1=acc,
                    op0=ALU.mult, op1=ALU.add)
            out_dma(b, jc, tcc, acc)

        pe_state = {'big': None}

        def agg_pe(b, jc, tcc):
            Z = z_tiles[b]
            Zw = Z if tcc == 0 else Z[:, :, 512:2 * L]
            i = pe_state.setdefault('i', 0)
            pe_state['i'] = i + 1
            if i % 2 == 0:
                pe_state['big'] = ps_big.tile([128, L], F32, tag="pa",
                                              name=f"apsb{b}_{jc}_{tcc}")
            big = pe_state['big']
            ps = big[:, 0:512] if i % 2 == 0 else big[:, 512:L]
            for k in range(TOPK):
                nc.tensor.matmul(ps, lhsT=wIs[b][k],
                                 rhs=Zw[:, jc, ds(tau_t[k], 512)],
                                 start=(k == 0), stop=(k == TOPK - 1),
                                 skip_group_check=True)
            acc = out_pool.tile([128, 512], BF16, tag="pacc",
                                name=f"pacc{b}_{jc}_{tcc}")
            nc.scalar.copy(acc, ps)
            out_dma(b, jc, tcc, acc)

        for eng_name, (b, jc, tcc) in zip(AGG_PAT, tiles):
            if eng_name == 'pe':
                agg_pe(b, jc, tcc)
            else:
                agg_dve(b, jc, tcc)

    nc.compile()
    return nc


def _get_nc():
    if "nc" not in _CACHE:
        _CACHE["nc"] = _build()
    return _CACHE["nc"]


def _run(inputs, trace=False, tmpdir=None):
    q_in = np.ascontiguousarray(inputs["q_in"], dtype=np.float32)
    k_in = np.ascontiguousarray(inputs["k_in"], dtype=np.float32)
    v_in = np.ascontiguousarray(inputs["v_in"], dtype=np.float32)
    Wq, Wk, Wv, Wo = inputs["Wq"], inputs["Wk"], inputs["Wv"], inputs["Wo"]
    bv, bo = inputs["bv"], inputs["bo"]

    A = ((Wq.astype(np.float64) @ Wk.astype(np.float64).T) / D).astype(bfloat16)
    Wc = (Wv.astype(np.float64) @ Wo.astype(np.float64)).astype(bfloat16)
    c_row = (bv.astype(np.float64) @ Wo.astype(np.float64) + bo).astype(np.float32)

    qT = np.ascontiguousarray(q_in.transpose(0, 2, 1).astype(bfloat16))
    kT = np.ascontiguousarray(k_in.transpose(0, 2, 1).astype(bfloat16))
    vT = np.ascontiguousarray(v_in.transpose(0, 2, 1).astype(bfloat16))
    I128 = np.eye(128, dtype=bfloat16)
    selrow = np.zeros((4, BLOC, 128), dtype=bfloat16)
    for b in range(BLOC):
        selrow[b, b, :] = 1
    selrow = selrow.reshape(4, BLOC * 128)

    nc = _get_nc()
    in_maps = []
    for c in range(NCORES):
        sl = slice(c * BLOC, (c + 1) * BLOC)
        in_maps.append({
            "qT": qT[sl], "kT": kT[sl], "vT": vT[sl],
            "A": A, "Wc": Wc, "I128": I128, "selrow": selrow,
        })
    res = run_bass_kernel_spmd(nc, in_maps, list(range(NCORES)),
                               trace=trace, tmpdir=tmpdir)
    outT = np.concatenate([np.asarray(r["outT"], dtype=np.float32)
                           for r in res.results], axis=0)  # (B, D, L)
    out = outT.transpose(0, 2, 1) + c_row[None, None, :]
    return np.ascontiguousarray(out, dtype=np.float32), res


def kernel(q_in, k_in, v_in, Wq, bq, Wk, bk, Wv, bv, Wo, bo):
    out, _ = _run(dict(q_in=q_in, k_in=k_in, v_in=v_in, Wq=Wq, bq=bq,
                       Wk=Wk, bk=bk, Wv=Wv, bv=bv, Wo=Wo, bo=bo))
    return out
